# revision 21
# baseline (speedup 1.0000x reference)
"""Trainium2 Bass kernel for nn_Attention_86698209837214.

Multi-head attention: out = softmax(q k^T / 8) v @ W_out + b_out with
B=4, N=2048, DIM=1024, H=16, Dh=64.

Sharding: 8 cores = (batch b in 0..3) x (head-half hh in 0..1); each core
computes 8 heads of one batch. Host pre-transposes x[b], slices weights,
casts everything to fp16 (PE streams fp16 at 2x the fp32r row rate; PSUM
accumulation stays fp32, so total rel err ~1e-3 << 2e-2 budget). Host sums
the two half-core partial outputs and adds b_out.

Device dataflow per core:
  1. v = x @ Wv in [n, c] layout, augmented per head with a ones column so
     attn@v also produces the softmax denominator in PSUM row 64.
  2. kT0/qT0 = (x @ Wk/Wq)^T [c, n]; the remaining 6 kT/qT c-tiles are
     generated inside the early attention chunks' PE idle slots (the
     attention steady state is ACT-bound: 16 exps of [128,1024] per chunk
     vs ~13us of PE work, so spare PE cycles absorb the projections).
  3. Attention chunks ordered hp-outer, ic-inner. Per chunk (hp, ic):
     8x{dots pair via K=64 row-positioned matmuls -> exp on ScalarE
     (scale=1/8 folded; logits ~N(0,1) so no max subtraction) -> attn@v
     lagging one step in PSUM}. Normalize: denominator row -> partition 0
     via DMA hop, fast reciprocal, partition_broadcast, DVE multiply; the
     s=1 half reaches aT partitions 64:127 via an SBUF-to-SBUF DMA.
  4. Out-projection accumulates all 4 head-pairs per output tile in PSUM
     (4x less DMA out + no host-side hp reduction); units are deferred one
     chunk so the epilogue latency hides, and interleave into the final
     hp round's PE slack.
"""

import sys

for _p in ("/opt/trn_rl_repo",):
    if _p not in sys.path:
        sys.path.append(_p)

from contextlib import ExitStack

import numpy as np

import concourse.bass as bass  # noqa: F401
import concourse.tile as tile
from concourse import bacc, mybir
from concourse.bass_utils import run_bass_kernel_spmd

F32 = mybir.dt.float32
F16 = mybir.dt.float16
AF = mybir.ActivationFunctionType

P = 128
NSEQ = 2048  # sequence length per batch
D = 1024  # model dim
CH = 512  # per-core head-dim width (8 heads x 64)
DH = 64
NPAIR = 4  # head pairs per core (c-tiles of 128)
NDT = D // P  # 8 d-tiles
NNT = NSEQ // P  # 16 n-tiles
NNC = NSEQ // 512  # 4 n-chunks
NJP = NNT // 2  # 8 j-tile pairs
SCALE = 0.125  # DIM_HEAD ** -0.5


def build_program():
    nc = bacc.Bacc("TRN2", target_bir_lowering=False, debug=False)

    xt = nc.dram_tensor("xt", [D, NSEQ], F16, kind="ExternalInput")
    wqkv = nc.dram_tensor("wqkv", [D, 3 * CH], F16, kind="ExternalInput")
    wout = nc.dram_tensor("wout", [CH, D], F16, kind="ExternalInput")
    ones_in = nc.dram_tensor("ones", [P, 1], F16, kind="ExternalInput")
    out = nc.dram_tensor("out", [NSEQ, D], F16, kind="ExternalOutput")

    xt_t = xt.ap().rearrange("(dt p) n -> dt p n", p=P)  # [8, 128, 2048]
    wqkv_t = wqkv.ap().rearrange("(dt p) c -> dt p c", p=P)  # [8, 128, 1536]
    wout_t = wout.ap().rearrange("(ct p) e -> ct p e", p=P)  # [4, 128, 1024]
    out_t = out.ap().rearrange("(nt p) e -> nt p e", p=P)  # [16, 128, 1024]

    with tile.TileContext(nc) as tc, ExitStack() as ctx:
        p_xt = ctx.enter_context(tc.tile_pool(name="p_xt", bufs=1))  # 32 KB/p
        p_w = ctx.enter_context(tc.tile_pool(name="p_w", bufs=1))  # 24 KB/p
        p_v = ctx.enter_context(tc.tile_pool(name="p_v", bufs=1))  # 16.3 KB/p
        p_qk = ctx.enter_context(tc.tile_pool(name="p_qk", bufs=1))  # 32 KB/p
        p_small = ctx.enter_context(tc.tile_pool(name="p_small", bufs=1))
        p_exp = ctx.enter_context(tc.tile_pool(name="p_exp", bufs=18))  # 36 KB/p
        p_aT = ctx.enter_context(tc.tile_pool(name="p_aT", bufs=1))  # 16 KB/p
        p_wout = ctx.enter_context(tc.tile_pool(name="p_wout", bufs=1))  # 8 KB/p
        p_den = ctx.enter_context(tc.tile_pool(name="p_den", bufs=2))
        p_recip = ctx.enter_context(tc.tile_pool(name="p_recip", bufs=2))
        p_bcast = ctx.enter_context(tc.tile_pool(name="p_bcast", bufs=4))
        p_ostage = ctx.enter_context(tc.tile_pool(name="p_ostage", bufs=3))

        # PSUM: dots 2x[128,1024] (4 banks) + av 3x[65,512] (3) + aux (1) = 8
        ps_mm = ctx.enter_context(tc.tile_pool(name="ps_mm", bufs=2, space="PSUM"))
        ps_av = ctx.enter_context(tc.tile_pool(name="ps_av", bufs=3, space="PSUM"))
        ps_aux = ctx.enter_context(tc.tile_pool(name="ps_aux", bufs=1, space="PSUM"))

        ones = p_small.tile([P, 1], F16, tag="ones")
        nc.sync.dma_start(out=ones, in_=ones_in.ap())
        # dummy exp pulls the ACT_TABLE_LOAD for the Exp set into the initial
        # DMA wait instead of the first real softmax tile
        warm = p_small.tile([P, 1], F32, tag="warm")
        nc.scalar.activation(out=warm, in_=ones, func=AF.Exp, scale=1.0)

        # ---- input DMAs: spread across all 5 engine queues (per-queue DMA
        # bandwidth is the prologue limiter; whole-tile transfers keep
        # per-partition lines at 4KB for full throughput) ----
        queues = [nc.sync, nc.scalar, nc.gpsimd]
        qi = 0

        def qdma(out_ap, in_ap):
            nonlocal qi
            queues[qi % len(queues)].dma_start(out=out_ap, in_=in_ap)
            qi += 1

        # ---- v_aug tiles (ones columns filled below via ScalarE, NOT via
        # broadcast DMA: a [P,8,1] scatter DMA costs ~10us on hardware) ----
        v_tiles = [
            p_v.tile([P, 8 * 65], F16, tag=f"v{nt}", name=f"v{nt}")
            for nt in range(NNT)
        ]

        # critical path first on every queue: kq0 needs only the ct=0 column
        # slice of wk/wq, so land those tiny pieces before everything else.
        # gpsimd only gets a small share (it also carries the per-chunk
        # epilogue hop DMAs and must drain early).
        wk_tiles = [
            p_w.tile([P, CH], F16, tag=f"wk{dt_i}", name=f"wk{dt_i}")
            for dt_i in range(NDT)
        ]
        wq_tiles = [
            p_w.tile([P, CH], F16, tag=f"wq{dt_i}", name=f"wq{dt_i}")
            for dt_i in range(NDT)
        ]
        for dt_i in range(NDT):
            qdma(wk_tiles[dt_i][:, 0:P], wqkv_t[dt_i][:, CH : CH + P])
            qdma(wq_tiles[dt_i][:, 0:P], wqkv_t[dt_i][:, 0:P])
        xt_tiles = []
        for dt_i in range(NDT):
            xt_tiles.append(
                p_xt.tile([P, NSEQ], F16, tag=f"xt{dt_i}", name=f"xt{dt_i}")
            )
            qdma(xt_tiles[dt_i], xt_t[dt_i])
        wv_tiles = []
        for dt_i in range(NDT):
            t = p_w.tile([P, CH], F16, tag=f"wv{dt_i}")
            # wv feeds the v-gen pieces right after kq0: keep off gpsimd
            queues[dt_i % 2].dma_start(out=t, in_=wqkv_t[dt_i][:, 2 * CH : 3 * CH])
            wv_tiles.append(t)
        for dt_i in range(NDT):
            queues[dt_i % 2].dma_start(
                out=wk_tiles[dt_i][:, P:CH], in_=wqkv_t[dt_i][:, CH + P : 2 * CH]
            )
        for dt_i in range(NDT):
            queues[dt_i % 2].dma_start(
                out=wq_tiles[dt_i][:, P:CH], in_=wqkv_t[dt_i][:, P:CH]
            )
        wout_tiles = []
        for ct in range(NPAIR):
            t = p_wout.tile([P, D], F16, tag=f"wout{ct}")
            queues[ct % 2].dma_start(out=t, in_=wout_t[ct])
            wout_tiles.append(t)

        # ---- kT/qT c-tiles; only hp=0 upfront, rest fill attention slack ----
        kT_tiles = [
            p_qk.tile([P, NSEQ], F16, tag=f"kT{ct}", name=f"kT{ct}")
            for ct in range(NPAIR)
        ]
        qT_tiles = [
            p_qk.tile([P, NSEQ], F16, tag=f"qT{ct}", name=f"qT{ct}")
            for ct in range(NPAIR)
        ]

        def emit_qk_piece(which, ct, ncs, pool, tag):
            w_tiles = wk_tiles if which == "k" else wq_tiles
            dst = (kT_tiles if which == "k" else qT_tiles)[ct]
            woff = ct * P
            acc = pool.tile([P, 512], F32, tag=tag)
            for dt_i in range(NDT):
                nc.tensor.matmul(
                    acc,
                    w_tiles[dt_i][:, woff : woff + P],
                    xt_tiles[dt_i][:, ncs * 512 : (ncs + 1) * 512],
                    start=(dt_i == 0),
                    stop=(dt_i == NDT - 1),
                )
            nc.vector.tensor_copy(dst[:, ncs * 512 : (ncs + 1) * 512], acc)

        for ncs in range(NNC):
            emit_qk_piece("k", 0, ncs, ps_mm, "mm")
        for ncs in range(NNC):
            emit_qk_piece("q", 0, ncs, ps_mm, "mm")

        def emit_dots_exp(hp, ic, jp):
            """one j-pair of dots + exp for chunk (hp, ic); returns the exp
            pair [half] each holding [s0 | s1] of j-tile 2jp+half"""
            i0 = ic * 512
            dots_tiles = [
                ps_mm.tile([P, 1024], F32, tag="mm", name=f"dots{h}")
                for h in range(2)
            ]
            for half in range(2):
                jtx = 2 * jp + half
                for s in range(2):
                    r0 = s * DH
                    nc.tensor.matmul(
                        dots_tiles[half][:, s * 512 : (s + 1) * 512],
                        kT_tiles[hp][r0 : r0 + DH, jtx * P : (jtx + 1) * P],
                        qT_tiles[hp][r0 : r0 + DH, i0 : i0 + 512],
                        start=True,
                        stop=True,
                        tile_position=(r0, 0),
                    )
            exp_tiles = []
            for h in range(2):
                e = p_exp.tile([P, 1024], F16, tag="exp")
                nc.scalar.activation(
                    out=e, in_=dots_tiles[h], func=AF.Exp, scale=SCALE
                )
                exp_tiles.append(e)
            return exp_tiles

        def emit_v_piece(nt):
            # v accumulators borrow the (still idle) ps_av ring: 3 slots
            # pipeline the PSUM->SBUF copies behind the next pieces' matmuls
            acc = ps_av.tile([P, 512], F32, tag="av", name="vacc")
            for dt_i in range(NDT):
                nc.tensor.matmul(
                    acc,
                    xt_tiles[dt_i][:, nt * P : (nt + 1) * P],
                    wv_tiles[dt_i],
                    start=(dt_i == 0),
                    stop=(dt_i == NDT - 1),
                )
            v_dst = v_tiles[nt].rearrange("p (h c) -> p h c", c=65)[:, :, 0:DH]
            nc.vector.tensor_copy(v_dst, acc.rearrange("p (h c) -> p h c", c=DH))

        # fill the v_aug ones columns on ScalarE: out = 0*in + 1 (any loaded
        # tile works as the dummy input)
        dummy_in = wk_tiles[0][:, 0:8].rearrange("p (a b) -> p a b", b=1)
        for nt in range(NNT):
            ones_view = v_tiles[nt].rearrange("p (h c) -> p h c", c=65)[:, :, 64:65]
            nc.scalar.activation(
                out=ones_view, in_=dummy_in, func=AF.Copy, bias=1.0, scale=0.0
            )

        # chunk (0,0)'s dots+exp run interleaved with v-gen: ScalarE gets a
        # full chunk of exp work while the PE generates v; the attn@v for
        # this chunk flushes in the main loop once v exists
        c0_exps = []
        for jp in range(NJP):
            c0_exps.append(emit_dots_exp(0, 0, jp))
            emit_v_piece(2 * jp)
            emit_v_piece(2 * jp + 1)

        aT_tiles = [
            p_aT.tile([P, NSEQ], F16, tag=f"aT{hp}", name=f"aT{hp}")
            for hp in range(NPAIR)
        ]

        def emit_outproj_unit(nt):
            o_ps = ps_mm.tile([P, D], F32, tag="mm")
            for ec in range(2):
                for hpp in range(NPAIR):
                    nc.tensor.matmul(
                        o_ps[:, ec * 512 : (ec + 1) * 512],
                        aT_tiles[hpp][:, nt * P : (nt + 1) * P],
                        wout_tiles[hpp][:, ec * 512 : (ec + 1) * 512],
                        start=(hpp == 0),
                        stop=(hpp == NPAIR - 1),
                    )
            o_sb = p_ostage.tile([P, D], F16, tag="o_sb")
            nc.vector.tensor_copy(o_sb, o_ps)
            # split across two queues: halves the exposed latency of the
            # final tiles' writeback
            nc.sync.dma_start(out=out_t[nt][:, 0:512], in_=o_sb[:, 0:512])
            nc.gpsimd.dma_start(out=out_t[nt][:, 512:D], in_=o_sb[:, 512:D])

        # ---- attention chunks, hp-outer / ic-inner ----
        for hp in range(NPAIR):
            # spare-PE work for this round: qk pieces (rounds 0-2, via the
            # aux PSUM bank so the dots ring keeps its 1-jp lookahead) or
            # out-projection units (round 3, via the dots ring: no qk
            # pieces compete there and PSUM has no spare banks)
            for ic in range(NNC):
                i0 = ic * 512
                av_ps = [
                    ps_av.tile([65, 512], F32, tag="av", name=f"av{hp}_{ic}_{s}")
                    for s in range(2)
                ]

                def emit_av(jp, exp_pair):
                    # exp_pair[half] holds [s0 | s1] of j-tile 2jp+half
                    for s in range(2):
                        sg = hp * 2 + s
                        for half in range(2):
                            jtx = 2 * jp + half
                            nc.tensor.matmul(
                                av_ps[s],
                                v_tiles[jtx][:, sg * 65 : sg * 65 + 65],
                                exp_pair[half][:, s * 512 : (s + 1) * 512],
                                start=(jp == 0 and half == 0),
                                stop=(jp == NJP - 1 and half == 1),
                            )

                if hp == 0 and ic == 0:
                    # dots+exp pre-emitted alongside v-gen; flush attn@v now
                    for jpx in range(NJP):
                        emit_av(jpx, c0_exps[jpx])
                else:
                    prev_exp = None
                    for jp in range(NJP):
                        exp_tiles = emit_dots_exp(hp, ic, jp)
                        if prev_exp is not None:
                            emit_av(jp - 1, prev_exp)
                        # spare-PE slots: deferred kT/qT pieces (rounds 0-2)
                        if hp == 0 and jp in (1, 3, 5):
                            piece = (ic - 1) * 3 + (jp - 1) // 2
                            if piece < 2 * NNC:
                                which = "k" if piece < NNC else "q"
                                emit_qk_piece(
                                    which, 1, piece % NNC, ps_aux, "aux"
                                )
                        elif hp in (1, 2) and jp in (2, 5):
                            piece = 2 * ic + (1 if jp == 5 else 0)
                            which = "k" if piece < NNC else "q"
                            emit_qk_piece(
                                which, hp + 1, piece % NNC, ps_aux, "aux"
                            )
                        if hp == NPAIR - 1 and ic > 0 and jp in (1, 3, 5, 7):
                            # out-projection for ic-1, one n-tile per slot
                            emit_outproj_unit(4 * (ic - 1) + (jp - 1) // 2)
                        prev_exp = exp_tiles
                    emit_av(NJP - 1, prev_exp)

                # epilogue: rows 0:64 = unnormalized attn-out, row 64 = denom.
                # Cross-partition moves go through DMA; Pool/DVE broadcast ops
                # only operate at partition base 0.
                den_hi = p_den.tile([65, 1024], F32, tag="den_hi")
                for s in range(2):
                    nc.vector.tensor_copy(
                        den_hi[64:65, s * 512 : (s + 1) * 512], av_ps[s][64:65, :]
                    )
                den_sb = p_den.tile([1, 1024], F32, tag="den_sb")
                nc.gpsimd.dma_start(out=den_sb, in_=den_hi[64:65, :])
                recip = p_recip.tile([1, 1024], F32, tag="recip")
                nc.vector.reciprocal_approx_fast(out=recip, in_=den_sb)
                bcast = []
                for s in range(2):
                    bc = p_bcast.tile([DH, 512], F32, tag="bcast", name=f"bc{s}")
                    nc.gpsimd.partition_broadcast(
                        out_ap=bc, in_ap=recip[:, s * 512 : (s + 1) * 512]
                    )
                    bcast.append(bc)
                nc.vector.tensor_mul(
                    aT_tiles[hp][0:DH, i0 : i0 + 512], av_ps[0][0:DH, :], bcast[0]
                )
                tmp = p_bcast.tile([DH, 512], F16, tag="tmp")
                nc.vector.tensor_mul(tmp, av_ps[1][0:DH, :], bcast[1])
                nc.gpsimd.dma_start(
                    out=aT_tiles[hp][DH:P, i0 : i0 + 512], in_=tmp
                )

        # final ic's out-projection (tail)
        for nt in range(4 * (NNC - 1), NNT):
            emit_outproj_unit(nt)

    nc.compile()
    return nc


_NC = None


def _get_program():
    global _NC
    if _NC is None:
        _NC = build_program()
    return _NC


INNER = 1024


def kernel(x, W_qkv, W_out, b_out):
    x = np.asarray(x, dtype=np.float32)
    W_qkv = np.asarray(W_qkv, dtype=np.float32)
    W_out = np.asarray(W_out, dtype=np.float32)
    b_out = np.asarray(b_out, dtype=np.float32)
    B = x.shape[0]

    nc = _get_program()
    in_maps = []
    for b in range(B):
        for hh in range(2):
            cs = hh * CH
            wq = W_qkv[:, cs : cs + CH]
            wk = W_qkv[:, INNER + cs : INNER + cs + CH]
            wv = W_qkv[:, 2 * INNER + cs : 2 * INNER + cs + CH]
            in_maps.append(
                {
                    "xt": np.ascontiguousarray(x[b].T).astype(np.float16),
                    "wqkv": np.concatenate([wq, wk, wv], axis=1).astype(np.float16),
                    "wout": np.ascontiguousarray(W_out[cs : cs + CH, :]).astype(
                        np.float16
                    ),
                    "ones": np.ones((P, 1), dtype=np.float16),
                }
            )
    res = run_bass_kernel_spmd(nc, in_maps, core_ids=list(range(8)))
    out = np.empty((B, NSEQ, D), dtype=np.float32)
    for b in range(B):
        out[b] = (
            res.results[2 * b]["out"].astype(np.float32)
            + res.results[2 * b + 1]["out"].astype(np.float32)
            + b_out
        )
    return out


# revision 22
# speedup vs baseline: 1.0095x; 1.0095x over previous
"""Trainium2 Bass kernel for nn_Attention_86698209837214.

Multi-head attention: out = softmax(q k^T / 8) v @ W_out + b_out with
B=4, N=2048, DIM=1024, H=16, Dh=64.

Sharding: 8 cores = (batch b in 0..3) x (head-half hh in 0..1); each core
computes 8 heads of one batch. Host pre-transposes x[b], slices weights,
casts everything to fp16 (PE streams fp16 at 2x the fp32r row rate; PSUM
accumulation stays fp32, so total rel err ~1e-3 << 2e-2 budget). Host sums
the two half-core partial outputs and adds b_out.

Device dataflow per core:
  1. v = x @ Wv in [n, c] layout, augmented per head with a ones column so
     attn@v also produces the softmax denominator in PSUM row 64.
  2. kT0/qT0 = (x @ Wk/Wq)^T [c, n]; the remaining 6 kT/qT c-tiles are
     generated inside the early attention chunks' PE idle slots (the
     attention steady state is ACT-bound: 16 exps of [128,1024] per chunk
     vs ~13us of PE work, so spare PE cycles absorb the projections).
  3. Attention chunks ordered hp-outer, ic-inner. Per chunk (hp, ic):
     8x{dots pair via K=64 row-positioned matmuls -> exp on ScalarE
     (scale=1/8 folded; logits ~N(0,1) so no max subtraction) -> attn@v
     lagging one step in PSUM}. Normalize: denominator row -> partition 0
     via DMA hop, fast reciprocal, partition_broadcast, DVE multiply; the
     s=1 half reaches aT partitions 64:127 via an SBUF-to-SBUF DMA.
  4. Out-projection accumulates all 4 head-pairs per output tile in PSUM
     (4x less DMA out + no host-side hp reduction); units are deferred one
     chunk so the epilogue latency hides, and interleave into the final
     hp round's PE slack.
"""

import sys

for _p in ("/opt/trn_rl_repo",):
    if _p not in sys.path:
        sys.path.append(_p)

from contextlib import ExitStack

import numpy as np

import concourse.bass as bass  # noqa: F401
import concourse.tile as tile
from concourse import bacc, mybir
from concourse.bass_utils import run_bass_kernel_spmd

F32 = mybir.dt.float32
F16 = mybir.dt.float16
AF = mybir.ActivationFunctionType

P = 128
NSEQ = 2048  # sequence length per batch
D = 1024  # model dim
CH = 512  # per-core head-dim width (8 heads x 64)
DH = 64
NPAIR = 4  # head pairs per core (c-tiles of 128)
NDT = D // P  # 8 d-tiles
NNT = NSEQ // P  # 16 n-tiles
NNC = NSEQ // 512  # 4 n-chunks
NJP = NNT // 2  # 8 j-tile pairs
SCALE = 0.125  # DIM_HEAD ** -0.5


def build_program():
    nc = bacc.Bacc("TRN2", target_bir_lowering=False, debug=False)

    xt = nc.dram_tensor("xt", [D, NSEQ], F16, kind="ExternalInput")
    wqkv = nc.dram_tensor("wqkv", [D, 3 * CH], F16, kind="ExternalInput")
    wout = nc.dram_tensor("wout", [CH, D], F16, kind="ExternalInput")
    ones_in = nc.dram_tensor("ones", [P, 1], F16, kind="ExternalInput")
    out = nc.dram_tensor("out", [NSEQ, D], F16, kind="ExternalOutput")

    xt_t = xt.ap().rearrange("(dt p) n -> dt p n", p=P)  # [8, 128, 2048]
    wqkv_t = wqkv.ap().rearrange("(dt p) c -> dt p c", p=P)  # [8, 128, 1536]
    wout_t = wout.ap().rearrange("(ct p) e -> ct p e", p=P)  # [4, 128, 1024]
    out_t = out.ap().rearrange("(nt p) e -> nt p e", p=P)  # [16, 128, 1024]

    with tile.TileContext(nc) as tc, ExitStack() as ctx:
        p_xt = ctx.enter_context(tc.tile_pool(name="p_xt", bufs=1))  # 32 KB/p
        p_w = ctx.enter_context(tc.tile_pool(name="p_w", bufs=1))  # 24 KB/p
        p_v = ctx.enter_context(tc.tile_pool(name="p_v", bufs=1))  # 16.3 KB/p
        p_qk = ctx.enter_context(tc.tile_pool(name="p_qk", bufs=1))  # 32 KB/p
        p_small = ctx.enter_context(tc.tile_pool(name="p_small", bufs=1))
        p_exp = ctx.enter_context(tc.tile_pool(name="p_exp", bufs=18))  # 36 KB/p
        p_aT = ctx.enter_context(tc.tile_pool(name="p_aT", bufs=1))  # 16 KB/p
        p_wout = ctx.enter_context(tc.tile_pool(name="p_wout", bufs=1))  # 8 KB/p
        p_den = ctx.enter_context(tc.tile_pool(name="p_den", bufs=2))
        p_recip = ctx.enter_context(tc.tile_pool(name="p_recip", bufs=2))
        p_bcast = ctx.enter_context(tc.tile_pool(name="p_bcast", bufs=4))
        p_ostage = ctx.enter_context(tc.tile_pool(name="p_ostage", bufs=3))

        # PSUM: dots 2x[128,1024] (4 banks) + av 3x[65,512] (3) + aux (1) = 8
        ps_mm = ctx.enter_context(tc.tile_pool(name="ps_mm", bufs=2, space="PSUM"))
        ps_av = ctx.enter_context(tc.tile_pool(name="ps_av", bufs=3, space="PSUM"))
        ps_aux = ctx.enter_context(tc.tile_pool(name="ps_aux", bufs=1, space="PSUM"))

        ones = p_small.tile([P, 1], F16, tag="ones")
        nc.sync.dma_start(out=ones, in_=ones_in.ap())
        # dummy exp pulls the ACT_TABLE_LOAD for the Exp set into the initial
        # DMA wait instead of the first real softmax tile
        warm = p_small.tile([P, 1], F32, tag="warm")
        nc.scalar.activation(out=warm, in_=ones, func=AF.Exp, scale=1.0)

        # ---- input DMAs: spread across all 5 engine queues (per-queue DMA
        # bandwidth is the prologue limiter; whole-tile transfers keep
        # per-partition lines at 4KB for full throughput) ----
        queues = [nc.sync, nc.scalar, nc.gpsimd]
        qi = 0

        def qdma(out_ap, in_ap):
            nonlocal qi
            queues[qi % len(queues)].dma_start(out=out_ap, in_=in_ap)
            qi += 1

        # ---- v_aug tiles (ones columns filled below via ScalarE, NOT via
        # broadcast DMA: a [P,8,1] scatter DMA costs ~10us on hardware) ----
        v_tiles = [
            p_v.tile([P, 8 * 65], F16, tag=f"v{nt}", name=f"v{nt}")
            for nt in range(NNT)
        ]

        # critical path first on every queue: kq0 needs only the ct=0 column
        # slice of wk/wq, so land those tiny pieces before everything else.
        # gpsimd only gets a small share (it also carries the per-chunk
        # epilogue hop DMAs and must drain early).
        wk_tiles = [
            p_w.tile([P, CH], F16, tag=f"wk{dt_i}", name=f"wk{dt_i}")
            for dt_i in range(NDT)
        ]
        wq_tiles = [
            p_w.tile([P, CH], F16, tag=f"wq{dt_i}", name=f"wq{dt_i}")
            for dt_i in range(NDT)
        ]
        for dt_i in range(NDT):
            qdma(wk_tiles[dt_i][:, 0:P], wqkv_t[dt_i][:, CH : CH + P])
            qdma(wq_tiles[dt_i][:, 0:P], wqkv_t[dt_i][:, 0:P])
        xt_tiles = []
        for dt_i in range(NDT):
            xt_tiles.append(
                p_xt.tile([P, NSEQ], F16, tag=f"xt{dt_i}", name=f"xt{dt_i}")
            )
            qdma(xt_tiles[dt_i], xt_t[dt_i])
        wv_tiles = []
        for dt_i in range(NDT):
            t = p_w.tile([P, CH], F16, tag=f"wv{dt_i}")
            # wv feeds the v-gen pieces right after kq0: keep off gpsimd
            queues[dt_i % 2].dma_start(out=t, in_=wqkv_t[dt_i][:, 2 * CH : 3 * CH])
            wv_tiles.append(t)
        for dt_i in range(NDT):
            queues[dt_i % 2].dma_start(
                out=wk_tiles[dt_i][:, P:CH], in_=wqkv_t[dt_i][:, CH + P : 2 * CH]
            )
        for dt_i in range(NDT):
            queues[dt_i % 2].dma_start(
                out=wq_tiles[dt_i][:, P:CH], in_=wqkv_t[dt_i][:, P:CH]
            )
        wout_tiles = []
        for ct in range(NPAIR):
            t = p_wout.tile([P, D], F16, tag=f"wout{ct}")
            queues[ct % 2].dma_start(out=t, in_=wout_t[ct])
            wout_tiles.append(t)

        # ---- kT/qT c-tiles; only hp=0 upfront, rest fill attention slack ----
        kT_tiles = [
            p_qk.tile([P, NSEQ], F16, tag=f"kT{ct}", name=f"kT{ct}")
            for ct in range(NPAIR)
        ]
        qT_tiles = [
            p_qk.tile([P, NSEQ], F16, tag=f"qT{ct}", name=f"qT{ct}")
            for ct in range(NPAIR)
        ]

        def emit_qk_piece(which, ct, ncs, pool, tag):
            w_tiles = wk_tiles if which == "k" else wq_tiles
            dst = (kT_tiles if which == "k" else qT_tiles)[ct]
            woff = ct * P
            acc = pool.tile([P, 512], F32, tag=tag)
            for dt_i in range(NDT):
                nc.tensor.matmul(
                    acc,
                    w_tiles[dt_i][:, woff : woff + P],
                    xt_tiles[dt_i][:, ncs * 512 : (ncs + 1) * 512],
                    start=(dt_i == 0),
                    stop=(dt_i == NDT - 1),
                )
            nc.vector.tensor_copy(dst[:, ncs * 512 : (ncs + 1) * 512], acc)

        for ncs in range(NNC):
            emit_qk_piece("k", 0, ncs, ps_mm, "mm")
        for ncs in range(NNC):
            emit_qk_piece("q", 0, ncs, ps_mm, "mm")

        def emit_dots_exp(hp, ic, jp):
            """one j-pair of dots + exp for chunk (hp, ic); returns the exp
            pair [half] each holding [s0 | s1] of j-tile 2jp+half"""
            i0 = ic * 512
            dots_tiles = [
                ps_mm.tile([P, 1024], F32, tag="mm", name=f"dots{h}")
                for h in range(2)
            ]
            for half in range(2):
                jtx = 2 * jp + half
                for s in range(2):
                    r0 = s * DH
                    nc.tensor.matmul(
                        dots_tiles[half][:, s * 512 : (s + 1) * 512],
                        kT_tiles[hp][r0 : r0 + DH, jtx * P : (jtx + 1) * P],
                        qT_tiles[hp][r0 : r0 + DH, i0 : i0 + 512],
                        start=True,
                        stop=True,
                        tile_position=(r0, 0),
                    )
            exp_tiles = []
            for h in range(2):
                e = p_exp.tile([P, 1024], F16, tag="exp")
                nc.scalar.activation(
                    out=e, in_=dots_tiles[h], func=AF.Exp, scale=SCALE
                )
                exp_tiles.append(e)
            return exp_tiles

        def emit_v_piece(nt):
            # v accumulators borrow the (still idle) ps_av ring: 3 slots
            # pipeline the PSUM->SBUF copies behind the next pieces' matmuls
            acc = ps_av.tile([P, 512], F32, tag="av", name="vacc")
            for dt_i in range(NDT):
                nc.tensor.matmul(
                    acc,
                    xt_tiles[dt_i][:, nt * P : (nt + 1) * P],
                    wv_tiles[dt_i],
                    start=(dt_i == 0),
                    stop=(dt_i == NDT - 1),
                )
            v_dst = v_tiles[nt].rearrange("p (h c) -> p h c", c=65)[:, :, 0:DH]
            nc.vector.tensor_copy(v_dst, acc.rearrange("p (h c) -> p h c", c=DH))

        # fill the v_aug ones columns on ScalarE: out = 0*in + 1 (any loaded
        # tile works as the dummy input)
        dummy_in = wk_tiles[0][:, 0:8].rearrange("p (a b) -> p a b", b=1)
        for nt in range(NNT):
            ones_view = v_tiles[nt].rearrange("p (h c) -> p h c", c=65)[:, :, 64:65]
            nc.scalar.activation(
                out=ones_view, in_=dummy_in, func=AF.Copy, bias=1.0, scale=0.0
            )

        # chunk (0,0)'s dots+exp run interleaved with v-gen: ScalarE gets a
        # full chunk of exp work while the PE generates v; the attn@v for
        # this chunk flushes in the main loop once v exists
        c0_exps = []
        for jp in range(NJP):
            c0_exps.append(emit_dots_exp(0, 0, jp))
            emit_v_piece(2 * jp)
            emit_v_piece(2 * jp + 1)

        aT_tiles = [
            p_aT.tile([P, NSEQ], F16, tag=f"aT{hp}", name=f"aT{hp}")
            for hp in range(NPAIR)
        ]

        def emit_outproj_unit(nt):
            o_ps = ps_mm.tile([P, D], F32, tag="mm")
            for ec in range(2):
                for hpp in range(NPAIR):
                    nc.tensor.matmul(
                        o_ps[:, ec * 512 : (ec + 1) * 512],
                        aT_tiles[hpp][:, nt * P : (nt + 1) * P],
                        wout_tiles[hpp][:, ec * 512 : (ec + 1) * 512],
                        start=(hpp == 0),
                        stop=(hpp == NPAIR - 1),
                    )
            o_sb = p_ostage.tile([P, D], F16, tag="o_sb")
            nc.vector.tensor_copy(o_sb, o_ps)
            nc.sync.dma_start(out=out_t[nt], in_=o_sb)

        # ---- attention chunks, hp-outer / ic-inner ----
        for hp in range(NPAIR):
            # spare-PE work for this round: qk pieces (rounds 0-2, via the
            # aux PSUM bank so the dots ring keeps its 1-jp lookahead) or
            # out-projection units (round 3, via the dots ring: no qk
            # pieces compete there and PSUM has no spare banks)
            for ic in range(NNC):
                i0 = ic * 512
                av_ps = [
                    ps_av.tile([65, 512], F32, tag="av", name=f"av{hp}_{ic}_{s}")
                    for s in range(2)
                ]

                def emit_av(jp, exp_pair):
                    # exp_pair[half] holds [s0 | s1] of j-tile 2jp+half
                    for s in range(2):
                        sg = hp * 2 + s
                        for half in range(2):
                            jtx = 2 * jp + half
                            nc.tensor.matmul(
                                av_ps[s],
                                v_tiles[jtx][:, sg * 65 : sg * 65 + 65],
                                exp_pair[half][:, s * 512 : (s + 1) * 512],
                                start=(jp == 0 and half == 0),
                                stop=(jp == NJP - 1 and half == 1),
                            )

                if hp == 0 and ic == 0:
                    # dots+exp pre-emitted alongside v-gen; flush attn@v now
                    for jpx in range(NJP):
                        emit_av(jpx, c0_exps[jpx])
                else:
                    prev_exp = None
                    for jp in range(NJP):
                        exp_tiles = emit_dots_exp(hp, ic, jp)
                        if prev_exp is not None:
                            emit_av(jp - 1, prev_exp)
                        # spare-PE slots: deferred kT/qT pieces (rounds 0-2)
                        if hp == 0 and jp in (1, 3, 5):
                            piece = (ic - 1) * 3 + (jp - 1) // 2
                            if piece < 2 * NNC:
                                which = "k" if piece < NNC else "q"
                                emit_qk_piece(
                                    which, 1, piece % NNC, ps_aux, "aux"
                                )
                        elif hp in (1, 2) and jp in (2, 5):
                            piece = 2 * ic + (1 if jp == 5 else 0)
                            which = "k" if piece < NNC else "q"
                            emit_qk_piece(
                                which, hp + 1, piece % NNC, ps_aux, "aux"
                            )
                        if hp == NPAIR - 1 and ic > 0 and jp in (1, 3, 5, 7):
                            # out-projection for ic-1, one n-tile per slot
                            emit_outproj_unit(4 * (ic - 1) + (jp - 1) // 2)
                        prev_exp = exp_tiles
                    emit_av(NJP - 1, prev_exp)

                # epilogue: rows 0:64 = unnormalized attn-out, row 64 = denom.
                # Cross-partition moves go through DMA; Pool/DVE broadcast ops
                # only operate at partition base 0.
                den_hi = p_den.tile([65, 1024], F32, tag="den_hi")
                for s in range(2):
                    nc.vector.tensor_copy(
                        den_hi[64:65, s * 512 : (s + 1) * 512], av_ps[s][64:65, :]
                    )
                den_sb = p_den.tile([1, 1024], F32, tag="den_sb")
                nc.gpsimd.dma_start(out=den_sb, in_=den_hi[64:65, :])
                recip = p_recip.tile([1, 1024], F32, tag="recip")
                nc.vector.reciprocal_approx_fast(out=recip, in_=den_sb)
                bcast = []
                for s in range(2):
                    bc = p_bcast.tile([DH, 512], F32, tag="bcast", name=f"bc{s}")
                    nc.gpsimd.partition_broadcast(
                        out_ap=bc, in_ap=recip[:, s * 512 : (s + 1) * 512]
                    )
                    bcast.append(bc)
                nc.vector.tensor_mul(
                    aT_tiles[hp][0:DH, i0 : i0 + 512], av_ps[0][0:DH, :], bcast[0]
                )
                tmp = p_bcast.tile([DH, 512], F16, tag="tmp")
                nc.vector.tensor_mul(tmp, av_ps[1][0:DH, :], bcast[1])
                nc.gpsimd.dma_start(
                    out=aT_tiles[hp][DH:P, i0 : i0 + 512], in_=tmp
                )

        # final ic's out-projection (tail)
        for nt in range(4 * (NNC - 1), NNT):
            emit_outproj_unit(nt)

    nc.compile()
    return nc


_NC = None


def _get_program():
    global _NC
    if _NC is None:
        _NC = build_program()
    return _NC


INNER = 1024


def kernel(x, W_qkv, W_out, b_out):
    x = np.asarray(x, dtype=np.float32)
    W_qkv = np.asarray(W_qkv, dtype=np.float32)
    W_out = np.asarray(W_out, dtype=np.float32)
    b_out = np.asarray(b_out, dtype=np.float32)
    B = x.shape[0]

    nc = _get_program()
    in_maps = []
    for b in range(B):
        for hh in range(2):
            cs = hh * CH
            wq = W_qkv[:, cs : cs + CH]
            wk = W_qkv[:, INNER + cs : INNER + cs + CH]
            wv = W_qkv[:, 2 * INNER + cs : 2 * INNER + cs + CH]
            in_maps.append(
                {
                    "xt": np.ascontiguousarray(x[b].T).astype(np.float16),
                    "wqkv": np.concatenate([wq, wk, wv], axis=1).astype(np.float16),
                    "wout": np.ascontiguousarray(W_out[cs : cs + CH, :]).astype(
                        np.float16
                    ),
                    "ones": np.ones((P, 1), dtype=np.float16),
                }
            )
    res = run_bass_kernel_spmd(nc, in_maps, core_ids=list(range(8)))
    out = np.empty((B, NSEQ, D), dtype=np.float32)
    for b in range(B):
        out[b] = (
            res.results[2 * b]["out"].astype(np.float32)
            + res.results[2 * b + 1]["out"].astype(np.float32)
            + b_out
        )
    return out
